# revision 40
# baseline (speedup 1.0000x reference)
"""Multi-head attention (B=2, N=2048, C=1024, H=16, D=64) on 8 Trainium2 cores.

Sharding: core c handles batch b=c//4 and heads [4r, 4r+4) where r=c%4
(batch-split across the two 4-core halves, head-split within a half).
After per-head attention, AllToAll collectives (one per local head, overlapped
with the remaining heads' compute) redistribute the attention output from
head-sharded to sequence-sharded: core g ends up with the full attn-T columns
for sequence rows [g*256, (g+1)*256) of BOTH batches and computes the output
projection for exactly those rows. The host only slices/casts/transposes
inputs and concatenates the outputs.

Matmul operands are bf16 by default (fp32 PSUM accumulation); softmax and
LayerNorm statistics are fp32. Set KERNEL_F32R=1 for float32r operands
(~13 mantissa bits) at higher PE cost.
"""
import os
import numpy as np

B, N, C = 2, 2048, 1024
H, D = 16, 64
LN_EPS = 1e-6
N_CORES = 8
HPC = 4          # heads per core
IH = 1024        # i-half width in the attention stage

_CACHE = {}


def _install_trace_shim():
    """Recreate the missing antenv.axon_hooks module so trace=True works."""
    import sys, types
    if "antenv.axon_hooks" in sys.modules:
        return
    try:
        import antenv
        mod = types.ModuleType("antenv.axon_hooks")
        mod._hook = None
        mod.set_axon_ntff_profile_hook = lambda h: setattr(mod, "_hook", h)
        mod.get_axon_ntff_profile_hook = lambda: mod._hook
        sys.modules["antenv.axon_hooks"] = mod
        antenv.axon_hooks = mod
        from trn_agent_boot.trn_boot import _ntff_profile_via_ctypes
        mod._hook = _ntff_profile_via_ctypes("/opt/axon/libaxon_pjrt.so")
    except Exception:
        pass


def _use_f32r():
    return os.environ.get("KERNEL_F32R", "0") == "1"


def _build():
    import concourse.bacc as bacc
    import concourse.bass as bass
    import concourse.tile as tile
    from concourse import mybir
    from concourse.masks import make_identity
    from contextlib import ExitStack

    f32 = mybir.dt.float32
    mdt = mybir.dt.float32r if _use_f32r() else mybir.dt.bfloat16

    AP = bass.AP
    nc = bacc.Bacc("TRN2", target_bir_lowering=False, debug=False,
                   num_devices=N_CORES)

    # ---- DRAM I/O (per-core shards prepared on host) ----
    xT_d = nc.dram_tensor("xT", [C, N], mdt, kind="ExternalInput")           # x[b].T
    wqk_d = nc.dram_tensor("wqk", [C, 512], mdt, kind="ExternalInput")       # [q cols | k cols]
    wv_d = nc.dram_tensor("wv", [C, 256], mdt, kind="ExternalInput")
    wproj_d = nc.dram_tensor("wproj", [C, C], mdt, kind="ExternalInput")
    bqk_d = nc.dram_tensor("bqk", [512], f32, kind="ExternalInput")
    bv_d = nc.dram_tensor("bv", [256], f32, kind="ExternalInput")
    bproj_d = nc.dram_tensor("bproj", [C], f32, kind="ExternalInput")
    lnsc_d = nc.dram_tensor("lnsc", [512], mdt, kind="ExternalInput")        # [q_scale x4 | k_scale x4]
    lnbi_d = nc.dram_tensor("lnbi", [512], mdt, kind="ExternalInput")
    out_d = nc.dram_tensor("out_part", [B, 256, C], f32, kind="ExternalOutput")

    def bcast(dram_handle, n_parts, free):
        ap = dram_handle.ap()
        return AP(tensor=ap.tensor, offset=0, ap=[[0, n_parts], [1, free]])

    groups = [[0, 1, 2, 3, 4, 5, 6, 7]]

    with tile.TileContext(nc) as tc:
        with ExitStack() as ctx:
            g = ctx.enter_context(tc.tile_pool(name="globals", bufs=1))
            dram = ctx.enter_context(tc.tile_pool(name="dram", bufs=1, space="DRAM"))

            # ---- constants ----
            identity_f32 = g.tile([128, 128], f32, tag="ident32")
            make_identity(nc, identity_f32)
            identity = g.tile([128, 128], mdt, tag="ident")
            nc.vector.tensor_copy(out=identity, in_=identity_f32)
            eps_t = g.tile([128, 1], f32, tag="eps")
            nc.vector.memset(eps_t, LN_EPS)
            bqk_bc = g.tile([128, 512], f32, tag="bqk")
            nc.sync.dma_start(out=bqk_bc, in_=bcast(bqk_d, 128, 512))
            bv_bc = g.tile([128, 256], f32, tag="bv")
            nc.sync.dma_start(out=bv_bc, in_=bcast(bv_d, 128, 256))
            lnsc_bc = g.tile([128, 512], mdt, tag="lnsc")
            nc.sync.dma_start(out=lnsc_bc, in_=bcast(lnsc_d, 128, 512))
            lnbi_bc = g.tile([128, 512], mdt, tag="lnbi")
            nc.sync.dma_start(out=lnbi_bc, in_=bcast(lnbi_d, 128, 512))
            bproj_bc = g.tile([128, C], f32, tag="bproj")
            nc.sync.dma_start(out=bproj_bc, in_=bcast(bproj_d, 128, C))

            # ---- persistent activations ----
            # q2/k2: [128, pair, n]; rows 0-63 = head 2p dims, 64-127 = head 2p+1
            q2 = g.tile([128, 2, N], mdt, tag="q2")
            k2 = g.tile([128, 2, N], mdt, tag="k2")
            # v with a ones column appended per head: [n-part, nt, head, 65]
            v_all = g.tile([128, 16, HPC, D + 1], mdt, tag="v_all")
            ones_t = g.tile([128, 16, HPC, 1], f32, tag="ones_t")
            nc.vector.memset(ones_t, 1.0)
            nc.vector.tensor_copy(out=v_all[:, :, :, D:D + 1], in_=ones_t)
            # unnormalized attn outT staging [64, head, n]
            outT = g.tile([64, HPC, N], mdt, tag="outT")

            # projection weights, prefetched during stage A (used only in stage C)
            wp_sb = g.tile([128, 8, C], mdt, tag="wp_sb")

            # per-head-pair collective buffers: slot s = 128 attnT rows for core s
            cc_in = [dram.tile([8, 128, 256], mdt, name=f"cc_in{p}") for p in range(2)]
            cc_out = [dram.tile([8, 128, 256], mdt, name=f"cc_out{p}") for p in range(2)]
            r_dram = nc.dram_tensor("r_stage", [8, IH], f32).ap()
            r_dram2 = nc.dram_tensor("r_stage2", [8, IH], f32).ap()

            # ================= Stage A: qkv + LN + transpose =================
            with ExitStack() as actx:
                sa = actx.enter_context(tc.tile_pool(name="stageA", bufs=1))
                qkp = actx.enter_context(tc.tile_pool(name="qk_pool", bufs=4))
                psA = actx.enter_context(tc.tile_pool(name="psA", bufs=2, space="PSUM"))
                psT = actx.enter_context(tc.tile_pool(name="psT", bufs=3, space="PSUM"))
                stp = actx.enter_context(tc.tile_pool(name="stats", bufs=4))

                xT = sa.tile([128, 8, N], mdt, tag="xT")
                wqk = sa.tile([128, 8, 512], mdt, tag="wqk")
                wv = sa.tile([128, 8, 256], mdt, tag="wv")
                for kc in range(8):
                    nc.sync.dma_start(
                        out=xT[:, kc, :],
                        in_=xT_d.ap()[kc * 128:(kc + 1) * 128, :])
                    nc.sync.dma_start(
                        out=wqk[:, kc, :],
                        in_=wqk_d.ap()[kc * 128:(kc + 1) * 128, :])
                    nc.sync.dma_start(
                        out=wv[:, kc, :],
                        in_=wv_d.ap()[kc * 128:(kc + 1) * 128, :])
                for kc in range(8):
                    nc.sync.dma_start(out=wp_sb[:, kc, :],
                                      in_=wproj_d.ap()[kc * 128:(kc + 1) * 128, :])

                for nt in range(16):
                    ps_qk = psA.tile([128, 512], f32, tag="ps_qk")
                    ps_v = psA.tile([128, 256], f32, tag="ps_v")
                    for kc in range(8):
                        nc.tensor.matmul(ps_qk, xT[:, kc, nt * 128:(nt + 1) * 128],
                                         wqk[:, kc, :], start=(kc == 0), stop=(kc == 7))
                    for kc in range(8):
                        nc.tensor.matmul(ps_v, xT[:, kc, nt * 128:(nt + 1) * 128],
                                         wv[:, kc, :], start=(kc == 0), stop=(kc == 7))

                    # biases
                    qk_sb = qkp.tile([128, 512], mdt, tag="qk_sb")
                    nc.vector.tensor_tensor(out=qk_sb, in0=ps_qk, in1=bqk_bc,
                                            op=mybir.AluOpType.add)
                    nc.vector.tensor_tensor(out=v_all[:, nt, :, 0:D],
                                            in0=ps_v.rearrange("p (h d) -> p h d", h=HPC),
                                            in1=bv_bc.rearrange("p (h d) -> p h d", h=HPC),
                                            op=mybir.AluOpType.add)

                    # LayerNorm per 64-col group (4 q heads + 4 k heads)
                    st8 = stp.tile([128, 8, 6], f32, tag="st8")
                    mv8 = stp.tile([128, 8, 2], f32, tag="mv8")
                    for gi in range(8):
                        nc.vector.bn_stats(out=st8[:, gi, :], in_=qk_sb[:, gi * D:(gi + 1) * D])
                        nc.vector.bn_aggr(out=mv8[:, gi, :], in_=st8[:, gi, :])
                    sd8 = stp.tile([128, 8], f32, tag="sd8")
                    nc.scalar.activation(out=sd8, in_=mv8[:, :, 1],
                                         func=mybir.ActivationFunctionType.Sqrt,
                                         bias=eps_t, scale=1.0)
                    rstd8 = stp.tile([128, 8], f32, tag="rstd8")
                    nc.vector.reciprocal(out=rstd8, in_=sd8)
                    for gi in range(8):
                        nc.vector.tensor_scalar(
                            out=qk_sb[:, gi * D:(gi + 1) * D],
                            in0=qk_sb[:, gi * D:(gi + 1) * D],
                            scalar1=mv8[:, gi, 0:1], scalar2=rstd8[:, gi:gi + 1],
                            op0=mybir.AluOpType.subtract, op1=mybir.AluOpType.mult)
                    nc.vector.tensor_tensor(out=qk_sb, in0=qk_sb, in1=lnsc_bc,
                                            op=mybir.AluOpType.mult)
                    nc.vector.tensor_tensor(out=qk_sb, in0=qk_sb, in1=lnbi_bc,
                                            op=mybir.AluOpType.add)

                    # transpose head pairs: cols [0:128)=q pair0, [128:256)=q pair1,
                    # [256:384)=k pair0, [384:512)=k pair1
                    for blk, dest in ((0, q2), (1, q2), (2, k2), (3, k2)):
                        pair = blk % 2
                        pt_ps = psT.tile([128, 128], mdt, tag="pt_ps")
                        nc.tensor.transpose(pt_ps, qk_sb[:, blk * 128:(blk + 1) * 128],
                                            identity)
                        nc.vector.tensor_copy(
                            out=dest[:, pair, nt * 128:(nt + 1) * 128], in_=pt_ps)

            # ================= Stage B: attention per head =================
            # The two i-halves (ih=0,1) are independent streams: while ACT
            # exponentiates one half's scores, PE works on the other half, so
            # the PE never starves on the exp dependency.
            with ExitStack() as bctx:
                pss = bctx.enter_context(tc.tile_pool(name="psS", bufs=1, space="PSUM"))
                pso = bctx.enter_context(tc.tile_pool(name="psO", bufs=1, space="PSUM"))
                ptp = bctx.enter_context(tc.tile_pool(name="pt_pool", bufs=4))
                nrm = bctx.enter_context(tc.tile_pool(name="nrm", bufs=2))

                for pair in range(2):
                    for ih in range(2):
                        ps_o = {}
                        for hp in range(2):
                            ps_o[hp] = pso.tile([65, IH], f32, tag=f"ps_o{hp}",
                                                name=f"ps_o{pair}_{ih}_{hp}")
                        for jt in range(16):
                            pts = {}
                            ps_s = {}
                            for hp in range(2):
                                ps_s[hp] = pss.tile([128, IH], f32, tag=f"ps_s{hp}",
                                                    name=f"ps_s{pair}_{ih}_{hp}_{jt}")
                            # adjacent matmuls on row groups 0-63 / 64-127 run
                            # concurrently in the PE sub-arrays
                            for icc in range(2):
                                for hp in range(2):
                                    po = hp * 64
                                    nc.tensor.matmul(
                                        ps_s[hp][:, icc * 512:(icc + 1) * 512],
                                        k2[po:po + 64, pair, jt * 128:(jt + 1) * 128],
                                        q2[po:po + 64, pair,
                                           ih * IH + icc * 512: ih * IH + (icc + 1) * 512],
                                        start=True, stop=True)
                            for hp in range(2):
                                pt = ptp.tile([128, IH], mdt, tag=f"pt{hp}",
                                              name=f"pt{pair}_{ih}_{hp}_{jt}")
                                nc.scalar.activation(out=pt, in_=ps_s[hp],
                                                     func=mybir.ActivationFunctionType.Exp,
                                                     scale=0.125)
                                pts[hp] = pt
                            for icc in range(2):
                                for hp in range(2):
                                    nc.tensor.matmul(
                                        ps_o[hp][:, icc * 512:(icc + 1) * 512],
                                        v_all[:, jt, 2 * pair + hp, :],
                                        pts[hp][:, icc * 512:(icc + 1) * 512],
                                        start=(jt == 0), stop=(jt == 15))

                        for hp in range(2):
                            h = 2 * pair + hp
                            # evacuate PSUM first so the accumulator frees early;
                            # rows 0-63 = unnormalized out, row 64 = sumexp
                            oe = nrm.tile([65, IH], f32, tag="oe")
                            nc.vector.tensor_copy(out=oe, in_=ps_o[hp])
                            nc.sync.dma_start(out=r_dram[2 * h + ih:2 * h + ih + 1, :],
                                              in_=oe[64:65, :])
                            r128 = nrm.tile([128, IH // 128], f32, tag="r128")
                            nc.sync.dma_start(
                                out=r128,
                                in_=r_dram[2 * h + ih, :].rearrange("(p t) -> p t", p=128))
                            nc.vector.reciprocal(out=r128, in_=r128)
                            nc.sync.dma_start(
                                out=r_dram2[2 * h + ih, :].rearrange("(p t) -> p t", p=128),
                                in_=r128)
                            r_slot = r_dram2[2 * h + ih, :]
                            r_bc = nrm.tile([64, IH], f32, tag="r_bc")
                            nc.sync.dma_start(
                                out=r_bc,
                                in_=AP(tensor=r_slot.tensor, offset=r_slot.offset,
                                       ap=[[0, 64], [1, IH]]))
                            nc.vector.tensor_tensor(out=outT[:, h, ih * IH:(ih + 1) * IH],
                                                    in0=oe[0:64, :], in1=r_bc,
                                                    op=mybir.AluOpType.mult)
                            # ship to pair collective input: slots 4*ih..4*ih+3,
                            # row block hp
                            nc.sync.dma_start(
                                out=cc_in[pair][4 * ih:4 * ih + 4,
                                                hp * 64:(hp + 1) * 64, :]
                                    .rearrange("s d i -> d s i"),
                                in_=outT[:, h, ih * IH:(ih + 1) * IH]
                                    .rearrange("d (s i) -> d s i", s=4))

                    # pair complete -> overlap its AllToAll with the next pair
                    nc.gpsimd.collective_compute(
                        "AllToAll", mybir.AluOpType.bypass, replica_groups=groups,
                        ins=[cc_in[pair].opt()], outs=[cc_out[pair].opt()])

            # ================= Stage C: projection =================
            with ExitStack() as cctx:
                atp = cctx.enter_context(tc.tile_pool(name="at_pool", bufs=3))
                psP = cctx.enter_context(tc.tile_pool(name="psP", bufs=1, space="PSUM"))
                oup = cctx.enter_context(tc.tile_pool(name="out_pool", bufs=3))

                # attnT rows for chunk kc = global heads 2kc, 2kc+1 of batch bb;
                # head g lives in cc_out[g % 4] slot (4*bb + g // 4)
                ps_list = {}
                for bb in range(B):
                    for mt in range(2):
                        for nk in range(2):
                            ps_p = psP.tile([128, 512], f32, tag=f"ps_p{bb}{mt}{nk}")
                            ps_list[(bb, mt, nk)] = ps_p
                # kc order consumes per-head collectives as they land:
                # chunk kc touches local heads {2kc%4, (2kc+1)%4}
                kc_order = [0, 2, 4, 6, 1, 3, 5, 7]
                for ki, kc in enumerate(kc_order):
                    wp_t = wp_sb[:, kc, :]
                    for bb in range(B):
                        at_t = atp.tile([128, 256], mdt, tag="at_t")
                        for half, gh in enumerate((2 * kc, 2 * kc + 1)):
                            lh = gh % 4  # local head on the source core
                            nc.sync.dma_start(
                                out=at_t[half * 64:(half + 1) * 64, :],
                                in_=cc_out[lh // 2][4 * bb + gh // 4,
                                                    (lh % 2) * 64:(lh % 2 + 1) * 64, :])
                        for mt in range(2):
                            for nk in range(2):
                                nc.tensor.matmul(
                                    ps_list[(bb, mt, nk)],
                                    at_t[:, mt * 128:(mt + 1) * 128],
                                    wp_t[:, nk * 512:(nk + 1) * 512],
                                    start=(ki == 0), stop=(ki == 7))
                for bb in range(B):
                    for mt in range(2):
                        o_sb = oup.tile([128, C], f32, tag="o_sb")
                        for nk in range(2):
                            nc.vector.tensor_tensor(
                                out=o_sb[:, nk * 512:(nk + 1) * 512],
                                in0=ps_list[(bb, mt, nk)],
                                in1=bproj_bc[:, nk * 512:(nk + 1) * 512],
                                op=mybir.AluOpType.add)
                        nc.sync.dma_start(
                            out=out_d.ap()[bb, mt * 128:(mt + 1) * 128, :], in_=o_sb)

    nc.compile()
    return nc


def kernel(**inputs):
    from concourse.bass_utils import run_bass_kernel_spmd
    import ml_dtypes

    trace = os.environ.get("KERNEL_TRACE", "0") == "1"
    if trace:
        _install_trace_shim()

    key = "nc_f32r" if _use_f32r() else "nc_bf16"
    if key not in _CACHE:
        _CACHE[key] = _build()
    nc = _CACHE[key]

    mnp = np.float32 if _use_f32r() else ml_dtypes.bfloat16

    x = np.asarray(inputs["x"], dtype=np.float32)
    w_qkv = np.asarray(inputs["w_qkv"], dtype=np.float32)
    b_qkv = np.asarray(inputs["b_qkv"], dtype=np.float32)
    w_proj = np.asarray(inputs["w_proj"], dtype=np.float32)
    b_proj = np.asarray(inputs["b_proj"], dtype=np.float32)
    q_scale = np.asarray(inputs["q_scale"], dtype=np.float32)
    q_bias = np.asarray(inputs["q_bias"], dtype=np.float32)
    k_scale = np.asarray(inputs["k_scale"], dtype=np.float32)
    k_bias = np.asarray(inputs["k_bias"], dtype=np.float32)

    lnsc = np.concatenate([np.tile(q_scale, HPC), np.tile(k_scale, HPC)])
    lnbi = np.concatenate([np.tile(q_bias, HPC), np.tile(k_bias, HPC)])
    wproj_m = np.ascontiguousarray(w_proj.astype(mnp))

    in_maps = []
    for c in range(N_CORES):
        b, r = divmod(c, 4)
        hs = slice(4 * r * D, 4 * r * D + 256)   # this core's head columns
        wqk = np.ascontiguousarray(np.concatenate(
            [w_qkv[:, 0 * C:][:, hs], w_qkv[:, 1 * C:][:, hs]], axis=1).astype(mnp))
        wv = np.ascontiguousarray(w_qkv[:, 2 * C:][:, hs].astype(mnp))
        bqk = np.concatenate([b_qkv[0 * C:][hs], b_qkv[1 * C:][hs]])
        bv = np.ascontiguousarray(b_qkv[2 * C:][hs])
        in_maps.append({
            "xT": np.ascontiguousarray(x[b].T.astype(mnp)),
            "wqk": wqk, "wv": wv, "wproj": wproj_m,
            "bqk": bqk, "bv": bv, "bproj": b_proj,
            "lnsc": lnsc.astype(mnp), "lnbi": lnbi.astype(mnp),
        })

    res = run_bass_kernel_spmd(nc, in_maps, core_ids=list(range(N_CORES)),
                               trace=trace)
    _CACHE["last_result"] = res

    out = np.empty((B, N, C), dtype=np.float32)
    for c in range(N_CORES):
        out[:, c * 256:(c + 1) * 256, :] = res.results[c]["out_part"]
    return out


# revision 41
# speedup vs baseline: 1.0431x; 1.0431x over previous
"""Multi-head attention (B=2, N=2048, C=1024, H=16, D=64) on 8 Trainium2 cores.

Sharding: core c handles batch b=c//4 and heads [4r, 4r+4) where r=c%4
(batch-split across the two 4-core halves, head-split within a half).
After per-head attention, AllToAll collectives (one per local head, overlapped
with the remaining heads' compute) redistribute the attention output from
head-sharded to sequence-sharded: core g ends up with the full attn-T columns
for sequence rows [g*256, (g+1)*256) of BOTH batches and computes the output
projection for exactly those rows. The host only slices/casts/transposes
inputs and concatenates the outputs.

Matmul operands are bf16 by default (fp32 PSUM accumulation); softmax and
LayerNorm statistics are fp32. Set KERNEL_F32R=1 for float32r operands
(~13 mantissa bits) at higher PE cost.
"""
import os
import numpy as np

B, N, C = 2, 2048, 1024
H, D = 16, 64
LN_EPS = 1e-6
N_CORES = 8
HPC = 4          # heads per core
IH = 1024        # i-half width in the attention stage

_CACHE = {}


def _install_trace_shim():
    """Recreate the missing antenv.axon_hooks module so trace=True works."""
    import sys, types
    if "antenv.axon_hooks" in sys.modules:
        return
    try:
        import antenv
        mod = types.ModuleType("antenv.axon_hooks")
        mod._hook = None
        mod.set_axon_ntff_profile_hook = lambda h: setattr(mod, "_hook", h)
        mod.get_axon_ntff_profile_hook = lambda: mod._hook
        sys.modules["antenv.axon_hooks"] = mod
        antenv.axon_hooks = mod
        from trn_agent_boot.trn_boot import _ntff_profile_via_ctypes
        mod._hook = _ntff_profile_via_ctypes("/opt/axon/libaxon_pjrt.so")
    except Exception:
        pass


def _use_f32r():
    return os.environ.get("KERNEL_F32R", "0") == "1"


def _build():
    import concourse.bacc as bacc
    import concourse.bass as bass
    import concourse.tile as tile
    from concourse import mybir
    from concourse.masks import make_identity
    from contextlib import ExitStack

    f32 = mybir.dt.float32
    mdt = mybir.dt.float32r if _use_f32r() else mybir.dt.bfloat16

    AP = bass.AP
    nc = bacc.Bacc("TRN2", target_bir_lowering=False, debug=False,
                   num_devices=N_CORES)

    # ---- DRAM I/O (per-core shards prepared on host) ----
    xT_d = nc.dram_tensor("xT", [C, N], mdt, kind="ExternalInput")           # x[b].T
    wqk_d = nc.dram_tensor("wqk", [C, 512], mdt, kind="ExternalInput")       # [q cols | k cols]
    wv_d = nc.dram_tensor("wv", [C, 256], mdt, kind="ExternalInput")
    wproj_d = nc.dram_tensor("wproj", [C, C], mdt, kind="ExternalInput")
    bqk_d = nc.dram_tensor("bqk", [512], f32, kind="ExternalInput")
    bv_d = nc.dram_tensor("bv", [256], f32, kind="ExternalInput")
    bproj_d = nc.dram_tensor("bproj", [C], f32, kind="ExternalInput")
    lnsc_d = nc.dram_tensor("lnsc", [512], mdt, kind="ExternalInput")        # [q_scale x4 | k_scale x4]
    lnbi_d = nc.dram_tensor("lnbi", [512], mdt, kind="ExternalInput")
    out_d = nc.dram_tensor("out_part", [B, 256, C], f32, kind="ExternalOutput")

    def bcast(dram_handle, n_parts, free):
        ap = dram_handle.ap()
        return AP(tensor=ap.tensor, offset=0, ap=[[0, n_parts], [1, free]])

    groups = [[0, 1, 2, 3, 4, 5, 6, 7]]

    with tile.TileContext(nc) as tc:
        with ExitStack() as ctx:
            g = ctx.enter_context(tc.tile_pool(name="globals", bufs=1))
            dram = ctx.enter_context(tc.tile_pool(name="dram", bufs=1, space="DRAM"))

            # ---- constants ----
            identity_f32 = g.tile([128, 128], f32, tag="ident32")
            make_identity(nc, identity_f32)
            identity = g.tile([128, 128], mdt, tag="ident")
            nc.vector.tensor_copy(out=identity, in_=identity_f32)
            eps_t = g.tile([128, 1], f32, tag="eps")
            nc.vector.memset(eps_t, LN_EPS)
            bqk_bc = g.tile([128, 512], f32, tag="bqk")
            nc.sync.dma_start(out=bqk_bc, in_=bcast(bqk_d, 128, 512))
            bv_bc = g.tile([128, 256], f32, tag="bv")
            nc.sync.dma_start(out=bv_bc, in_=bcast(bv_d, 128, 256))
            lnsc_bc = g.tile([128, 512], mdt, tag="lnsc")
            nc.sync.dma_start(out=lnsc_bc, in_=bcast(lnsc_d, 128, 512))
            lnbi_bc = g.tile([128, 512], mdt, tag="lnbi")
            nc.sync.dma_start(out=lnbi_bc, in_=bcast(lnbi_d, 128, 512))
            bproj_bc = g.tile([128, C], f32, tag="bproj")
            nc.sync.dma_start(out=bproj_bc, in_=bcast(bproj_d, 128, C))

            # ---- persistent activations ----
            # q2/k2: [128, pair, n]; rows 0-63 = head 2p dims, 64-127 = head 2p+1
            q2 = g.tile([128, 2, N], mdt, tag="q2")
            k2 = g.tile([128, 2, N], mdt, tag="k2")
            # v with a ones column appended per head: [n-part, nt, head, 65]
            v_all = g.tile([128, 16, HPC, D + 1], mdt, tag="v_all")
            ones_t = g.tile([128, 16, HPC, 1], f32, tag="ones_t")
            nc.vector.memset(ones_t, 1.0)
            nc.vector.tensor_copy(out=v_all[:, :, :, D:D + 1], in_=ones_t)
            # unnormalized attn outT staging [64, head, n]
            outT = g.tile([64, HPC, N], mdt, tag="outT")

            # projection weights, prefetched during stage A (used only in stage C)
            wp_sb = g.tile([128, 8, C], mdt, tag="wp_sb")

            # per-head-pair collective buffers: slot s = 128 attnT rows for core s
            cc_in = [dram.tile([8, 128, 256], mdt, name=f"cc_in{p}") for p in range(2)]
            cc_out = [dram.tile([8, 128, 256], mdt, name=f"cc_out{p}") for p in range(2)]
            r_dram = nc.dram_tensor("r_stage", [8, IH], f32).ap()
            r_dram2 = nc.dram_tensor("r_stage2", [8, IH], f32).ap()

            # ================= Stage A: qkv + LN + transpose =================
            with ExitStack() as actx:
                sa = actx.enter_context(tc.tile_pool(name="stageA", bufs=1))
                qkp = actx.enter_context(tc.tile_pool(name="qk_pool", bufs=3))
                psA = actx.enter_context(tc.tile_pool(name="psA", bufs=3, space="PSUM"))
                psT = actx.enter_context(tc.tile_pool(name="psT", bufs=2, space="PSUM"))
                stp = actx.enter_context(tc.tile_pool(name="stats", bufs=3))

                xT = sa.tile([128, 8, N], mdt, tag="xT")
                wqk = sa.tile([128, 8, 512], mdt, tag="wqk")
                wv = sa.tile([128, 8, 256], mdt, tag="wv")
                for kc in range(8):
                    nc.sync.dma_start(
                        out=xT[:, kc, :],
                        in_=xT_d.ap()[kc * 128:(kc + 1) * 128, :])
                    nc.sync.dma_start(
                        out=wqk[:, kc, :],
                        in_=wqk_d.ap()[kc * 128:(kc + 1) * 128, :])
                    nc.sync.dma_start(
                        out=wv[:, kc, :],
                        in_=wv_d.ap()[kc * 128:(kc + 1) * 128, :])
                for kc in range(8):
                    nc.sync.dma_start(out=wp_sb[:, kc, :],
                                      in_=wproj_d.ap()[kc * 128:(kc + 1) * 128, :])

                for nt in range(16):
                    ps_qk = psA.tile([128, 512], f32, tag="ps_qk")
                    ps_v = psA.tile([128, 256], f32, tag="ps_v")
                    for kc in range(8):
                        nc.tensor.matmul(ps_qk, xT[:, kc, nt * 128:(nt + 1) * 128],
                                         wqk[:, kc, :], start=(kc == 0), stop=(kc == 7))
                    for kc in range(8):
                        nc.tensor.matmul(ps_v, xT[:, kc, nt * 128:(nt + 1) * 128],
                                         wv[:, kc, :], start=(kc == 0), stop=(kc == 7))

                    # biases
                    qk_sb = qkp.tile([128, 512], mdt, tag="qk_sb")
                    nc.vector.tensor_tensor(out=qk_sb, in0=ps_qk, in1=bqk_bc,
                                            op=mybir.AluOpType.add)
                    nc.vector.tensor_tensor(out=v_all[:, nt, :, 0:D],
                                            in0=ps_v.rearrange("p (h d) -> p h d", h=HPC),
                                            in1=bv_bc.rearrange("p (h d) -> p h d", h=HPC),
                                            op=mybir.AluOpType.add)

                    # LayerNorm per 64-col group (4 q heads + 4 k heads)
                    st8 = stp.tile([128, 8, 6], f32, tag="st8")
                    mv8 = stp.tile([128, 8, 2], f32, tag="mv8")
                    for gi in range(8):
                        nc.vector.bn_stats(out=st8[:, gi, :], in_=qk_sb[:, gi * D:(gi + 1) * D])
                        nc.vector.bn_aggr(out=mv8[:, gi, :], in_=st8[:, gi, :])
                    sd8 = stp.tile([128, 8], f32, tag="sd8")
                    nc.scalar.activation(out=sd8, in_=mv8[:, :, 1],
                                         func=mybir.ActivationFunctionType.Sqrt,
                                         bias=eps_t, scale=1.0)
                    rstd8 = stp.tile([128, 8], f32, tag="rstd8")
                    nc.vector.reciprocal(out=rstd8, in_=sd8)
                    for gi in range(8):
                        nc.vector.tensor_scalar(
                            out=qk_sb[:, gi * D:(gi + 1) * D],
                            in0=qk_sb[:, gi * D:(gi + 1) * D],
                            scalar1=mv8[:, gi, 0:1], scalar2=rstd8[:, gi:gi + 1],
                            op0=mybir.AluOpType.subtract, op1=mybir.AluOpType.mult)
                    nc.vector.tensor_tensor(out=qk_sb, in0=qk_sb, in1=lnsc_bc,
                                            op=mybir.AluOpType.mult)
                    nc.vector.tensor_tensor(out=qk_sb, in0=qk_sb, in1=lnbi_bc,
                                            op=mybir.AluOpType.add)

                    # transpose head pairs: cols [0:128)=q pair0, [128:256)=q pair1,
                    # [256:384)=k pair0, [384:512)=k pair1
                    for blk, dest in ((0, q2), (1, q2), (2, k2), (3, k2)):
                        pair = blk % 2
                        pt_ps = psT.tile([128, 128], mdt, tag="pt_ps")
                        nc.tensor.transpose(pt_ps, qk_sb[:, blk * 128:(blk + 1) * 128],
                                            identity)
                        nc.vector.tensor_copy(
                            out=dest[:, pair, nt * 128:(nt + 1) * 128], in_=pt_ps)

            # ================= Stage B: attention per head =================
            # The two i-halves (ih=0,1) are independent streams: while ACT
            # exponentiates one half's scores, PE works on the other half, so
            # the PE never starves on the exp dependency.
            with ExitStack() as bctx:
                pss = bctx.enter_context(tc.tile_pool(name="psS", bufs=1, space="PSUM"))
                pso = bctx.enter_context(tc.tile_pool(name="psO", bufs=1, space="PSUM"))
                ptp = bctx.enter_context(tc.tile_pool(name="pt_pool", bufs=4))
                nrm = bctx.enter_context(tc.tile_pool(name="nrm", bufs=2))

                for pair in range(2):
                    for ih in range(2):
                        ps_o = {}
                        for hp in range(2):
                            ps_o[hp] = pso.tile([65, IH], f32, tag=f"ps_o{hp}",
                                                name=f"ps_o{pair}_{ih}_{hp}")
                        for jt in range(16):
                            pts = {}
                            ps_s = {}
                            for hp in range(2):
                                ps_s[hp] = pss.tile([128, IH], f32, tag=f"ps_s{hp}",
                                                    name=f"ps_s{pair}_{ih}_{hp}_{jt}")
                            # adjacent matmuls on row groups 0-63 / 64-127 run
                            # concurrently in the PE sub-arrays
                            for icc in range(2):
                                for hp in range(2):
                                    po = hp * 64
                                    nc.tensor.matmul(
                                        ps_s[hp][:, icc * 512:(icc + 1) * 512],
                                        k2[po:po + 64, pair, jt * 128:(jt + 1) * 128],
                                        q2[po:po + 64, pair,
                                           ih * IH + icc * 512: ih * IH + (icc + 1) * 512],
                                        start=True, stop=True)
                            for hp in range(2):
                                pt = ptp.tile([128, IH], mdt, tag=f"pt{hp}",
                                              name=f"pt{pair}_{ih}_{hp}_{jt}")
                                nc.scalar.activation(out=pt, in_=ps_s[hp],
                                                     func=mybir.ActivationFunctionType.Exp,
                                                     scale=0.125)
                                pts[hp] = pt
                            for icc in range(2):
                                for hp in range(2):
                                    nc.tensor.matmul(
                                        ps_o[hp][:, icc * 512:(icc + 1) * 512],
                                        v_all[:, jt, 2 * pair + hp, :],
                                        pts[hp][:, icc * 512:(icc + 1) * 512],
                                        start=(jt == 0), stop=(jt == 15))

                        for hp in range(2):
                            h = 2 * pair + hp
                            # evacuate PSUM first so the accumulator frees early;
                            # rows 0-63 = unnormalized out, row 64 = sumexp
                            oe = nrm.tile([65, IH], f32, tag="oe")
                            nc.vector.tensor_copy(out=oe, in_=ps_o[hp])
                            nc.sync.dma_start(out=r_dram[2 * h + ih:2 * h + ih + 1, :],
                                              in_=oe[64:65, :])
                            r128 = nrm.tile([128, IH // 128], f32, tag="r128")
                            nc.sync.dma_start(
                                out=r128,
                                in_=r_dram[2 * h + ih, :].rearrange("(p t) -> p t", p=128))
                            nc.vector.reciprocal(out=r128, in_=r128)
                            nc.sync.dma_start(
                                out=r_dram2[2 * h + ih, :].rearrange("(p t) -> p t", p=128),
                                in_=r128)
                            r_slot = r_dram2[2 * h + ih, :]
                            r_bc = nrm.tile([64, IH], f32, tag="r_bc")
                            nc.sync.dma_start(
                                out=r_bc,
                                in_=AP(tensor=r_slot.tensor, offset=r_slot.offset,
                                       ap=[[0, 64], [1, IH]]))
                            nc.vector.tensor_tensor(out=outT[:, h, ih * IH:(ih + 1) * IH],
                                                    in0=oe[0:64, :], in1=r_bc,
                                                    op=mybir.AluOpType.mult)
                            # ship to pair collective input: slots 4*ih..4*ih+3,
                            # row block hp
                            nc.sync.dma_start(
                                out=cc_in[pair][4 * ih:4 * ih + 4,
                                                hp * 64:(hp + 1) * 64, :]
                                    .rearrange("s d i -> d s i"),
                                in_=outT[:, h, ih * IH:(ih + 1) * IH]
                                    .rearrange("d (s i) -> d s i", s=4))

                    # pair complete -> overlap its AllToAll with the next pair
                    nc.gpsimd.collective_compute(
                        "AllToAll", mybir.AluOpType.bypass, replica_groups=groups,
                        ins=[cc_in[pair].opt()], outs=[cc_out[pair].opt()])

            # ================= Stage C: projection =================
            with ExitStack() as cctx:
                atp = cctx.enter_context(tc.tile_pool(name="at_pool", bufs=3))
                psP = cctx.enter_context(tc.tile_pool(name="psP", bufs=1, space="PSUM"))
                oup = cctx.enter_context(tc.tile_pool(name="out_pool", bufs=3))

                # attnT rows for chunk kc = global heads 2kc, 2kc+1 of batch bb;
                # head g lives in cc_out[g % 4] slot (4*bb + g // 4)
                ps_list = {}
                for bb in range(B):
                    for mt in range(2):
                        for nk in range(2):
                            ps_p = psP.tile([128, 512], f32, tag=f"ps_p{bb}{mt}{nk}")
                            ps_list[(bb, mt, nk)] = ps_p
                # kc order consumes per-head collectives as they land:
                # chunk kc touches local heads {2kc%4, (2kc+1)%4}
                kc_order = [0, 2, 4, 6, 1, 3, 5, 7]
                for ki, kc in enumerate(kc_order):
                    wp_t = wp_sb[:, kc, :]
                    for bb in range(B):
                        at_t = atp.tile([128, 256], mdt, tag="at_t")
                        for half, gh in enumerate((2 * kc, 2 * kc + 1)):
                            lh = gh % 4  # local head on the source core
                            nc.sync.dma_start(
                                out=at_t[half * 64:(half + 1) * 64, :],
                                in_=cc_out[lh // 2][4 * bb + gh // 4,
                                                    (lh % 2) * 64:(lh % 2 + 1) * 64, :])
                        for mt in range(2):
                            for nk in range(2):
                                nc.tensor.matmul(
                                    ps_list[(bb, mt, nk)],
                                    at_t[:, mt * 128:(mt + 1) * 128],
                                    wp_t[:, nk * 512:(nk + 1) * 512],
                                    start=(ki == 0), stop=(ki == 7))
                for bb in range(B):
                    for mt in range(2):
                        o_sb = oup.tile([128, C], f32, tag="o_sb")
                        for nk in range(2):
                            nc.vector.tensor_tensor(
                                out=o_sb[:, nk * 512:(nk + 1) * 512],
                                in0=ps_list[(bb, mt, nk)],
                                in1=bproj_bc[:, nk * 512:(nk + 1) * 512],
                                op=mybir.AluOpType.add)
                        nc.sync.dma_start(
                            out=out_d.ap()[bb, mt * 128:(mt + 1) * 128, :], in_=o_sb)

    nc.compile()
    return nc


def kernel(**inputs):
    from concourse.bass_utils import run_bass_kernel_spmd
    import ml_dtypes

    trace = os.environ.get("KERNEL_TRACE", "0") == "1"
    if trace:
        _install_trace_shim()

    key = "nc_f32r" if _use_f32r() else "nc_bf16"
    if key not in _CACHE:
        _CACHE[key] = _build()
    nc = _CACHE[key]

    mnp = np.float32 if _use_f32r() else ml_dtypes.bfloat16

    x = np.asarray(inputs["x"], dtype=np.float32)
    w_qkv = np.asarray(inputs["w_qkv"], dtype=np.float32)
    b_qkv = np.asarray(inputs["b_qkv"], dtype=np.float32)
    w_proj = np.asarray(inputs["w_proj"], dtype=np.float32)
    b_proj = np.asarray(inputs["b_proj"], dtype=np.float32)
    q_scale = np.asarray(inputs["q_scale"], dtype=np.float32)
    q_bias = np.asarray(inputs["q_bias"], dtype=np.float32)
    k_scale = np.asarray(inputs["k_scale"], dtype=np.float32)
    k_bias = np.asarray(inputs["k_bias"], dtype=np.float32)

    lnsc = np.concatenate([np.tile(q_scale, HPC), np.tile(k_scale, HPC)])
    lnbi = np.concatenate([np.tile(q_bias, HPC), np.tile(k_bias, HPC)])
    wproj_m = np.ascontiguousarray(w_proj.astype(mnp))

    in_maps = []
    for c in range(N_CORES):
        b, r = divmod(c, 4)
        hs = slice(4 * r * D, 4 * r * D + 256)   # this core's head columns
        wqk = np.ascontiguousarray(np.concatenate(
            [w_qkv[:, 0 * C:][:, hs], w_qkv[:, 1 * C:][:, hs]], axis=1).astype(mnp))
        wv = np.ascontiguousarray(w_qkv[:, 2 * C:][:, hs].astype(mnp))
        bqk = np.concatenate([b_qkv[0 * C:][hs], b_qkv[1 * C:][hs]])
        bv = np.ascontiguousarray(b_qkv[2 * C:][hs])
        in_maps.append({
            "xT": np.ascontiguousarray(x[b].T.astype(mnp)),
            "wqk": wqk, "wv": wv, "wproj": wproj_m,
            "bqk": bqk, "bv": bv, "bproj": b_proj,
            "lnsc": lnsc.astype(mnp), "lnbi": lnbi.astype(mnp),
        })

    res = run_bass_kernel_spmd(nc, in_maps, core_ids=list(range(N_CORES)),
                               trace=trace)
    _CACHE["last_result"] = res

    out = np.empty((B, N, C), dtype=np.float32)
    for c in range(N_CORES):
        out[:, c * 256:(c + 1) * 256, :] = res.results[c]["out_part"]
    return out
